# revision 22
# baseline (speedup 1.0000x reference)
"""Trainium2 Bass kernel for a 1-layer LSTM (B=2048, T=512, I=4, H=64) + FC (O=4).

Sharding: data-parallel over batch across 8 NeuronCores (256 examples/core);
the tiny LSTM/FC weights are replicated.

On-core layout: SBUF partitions carry hidden/gate rows, the free dimension
carries batch.  The 256 local examples form two groups of 128; the groups are
stacked in the partition dimension (group 0 -> rows 0-63, group 1 -> rows
64-127) so ScalarE/VectorE instructions run with all 128 lanes busy.

The recurrent state is a single tile hbuf[128, 128] (both groups' h stacked).
Per step the gate pre-activations are built by PSUM accumulation of two
matmuls per gate chunk:
  mm_x (start=True):  stat Wx2[10, 128]  x  x-slice[10, 128]  (bias+x part)
  mm_h (stop=True):   stat Wh2[128, 128] x  hbuf[128, 128]    (recurrent)
      Wh2 = blockdiag(W_hh_chunk^T, W_hh_chunk^T) so ONE matmul covers both
      groups; only 4 h-matmuls gate the step.
Then on ScalarE: sigmoid(i|f) (strided ACT across the two banks), tanh(g),
sigmoid(o) (in the ScalarE shadow), tanh(c); on VectorE: w = sf*c, u = si*tg,
c = u + w, h = so * tanh(c).

v3 changes vs the 1.247 ms baseline:
  - The whole input x is preloaded into SBUF once (xall, [40, T/4, 128]
    bf16 = 32 KB/partition), eliminating the per-step DMA and its
    sync-sequencer descriptor generation (~750 ns/step of queue work).
  - ScalarE "pad" ops (dummy sigmoids on PSUM scratch) are inserted before
    the two ACT instructions that otherwise start from an idle engine
    (sigmoid(i|f) and tanh(c)).  A gapped ACT start costs ~116 ns extra on
    TRN2 (read-write bubble); a pad sized to end just as the real op's
    dependency lands converts that to a back-to-back start.
"""

from contextlib import ExitStack

import numpy as np

import concourse.bass as bass
import concourse.tile as tile
from concourse import bacc, mybir
from concourse.bass_utils import run_bass_kernel_spmd

F32 = mybir.dt.float32
BF16 = mybir.dt.bfloat16
AF = mybir.ActivationFunctionType

H, I, O = 64, 4, 4
B, T_FULL = 2048, 512
NCORES = 8
BLOC = B // NCORES          # 256 examples per core
NG = 128                    # batch per group (2 groups per core)
KX = 2 * (1 + I)            # 10 rows of x-slice: [1; x_g0; 1; x_g1]
XFOLD = 2                   # timesteps folded into the xall partition dim (bases 0/64)

USE_BF16 = True

# PE issue order of the gate chunks (ids: 0=i, 1=f, 2=g, 3=o)
CHUNKS = (1, 0, 2, 3)

# ACT pad sizes (free-dim elements of the dummy sigmoid); 0 disables.
# pad1 runs between sigmoid(o) and tanh(c); pad2 between tanh(c) and the
# next step's sigmoid(i|f).
PAD1_FD = 0
PAD2_FD = 0
N_DUMMY = 4                 # fat PE matmuls stretching the stream to the h-deadline
DUMMY_FD = 512


def build_nc(T=T_FULL, use_bf16=None, pad1=None, pad2=None):
    if use_bf16 is None:
        use_bf16 = USE_BF16
    if pad1 is None:
        pad1 = PAD1_FD
    if pad2 is None:
        pad2 = PAD2_FD
    DT = BF16 if use_bf16 else F32
    assert T % XFOLD == 0
    TQ = T // XFOLD
    nc = bacc.Bacc(
        "TRN2",
        target_bir_lowering=False,
        debug=False,
        enable_asserts=False,
        num_devices=NCORES,
    )

    xq = nc.dram_tensor("xq", [128, TQ, NG], DT, kind="ExternalInput")
    wh2 = nc.dram_tensor("wh2", [2 * H, 4, 2 * H], DT, kind="ExternalInput")
    wx2 = nc.dram_tensor("wx2", [128, 4, 2 * H], DT, kind="ExternalInput")
    wfc2 = nc.dram_tensor("wfc2", [2 * H, 2 * O], DT, kind="ExternalInput")
    out = nc.dram_tensor("out", [2 * O, NG], F32, kind="ExternalOutput")

    with tile.TileContext(nc) as tc, ExitStack() as ctx:
        persist = ctx.enter_context(tc.tile_pool(name="persist", bufs=1))
        acts = ctx.enter_context(tc.tile_pool(name="acts", bufs=3))
        temps = ctx.enter_context(tc.tile_pool(name="temps", bufs=3))
        psum = ctx.enter_context(tc.tile_pool(name="psum", bufs=1, space="PSUM"))
        psum1 = ctx.enter_context(tc.tile_pool(name="psum1", bufs=1, space="PSUM"))
        psum2 = ctx.enter_context(tc.tile_pool(name="psum2", bufs=1, space="PSUM"))

        # Whole-input preload plus weights. xall is the big one; it goes on
        # the sync queue, the small weights on the ScalarE HWDGE queue.
        xall = persist.tile([128, TQ, NG], DT, tag="xall")
        nc.sync.dma_start(xall[:], xq[:])
        wh2_sb = persist.tile([2 * H, 4, 2 * H], DT, tag="wh2")
        nc.sync.dma_start(wh2_sb[:], wh2[:])
        # x-weights replicated at partition bases 0/32/64/96 so the matmul's
        # stationary base matches the xall slice base (PE tile-position rule).
        wx2_sb = persist.tile([128, 4, 2 * H], DT, tag="wx2")
        nc.scalar.dma_start(wx2_sb[:], wx2[:])
        wfc2_sb = persist.tile([2 * H, 2 * O], DT, tag="wfc2")
        nc.scalar.dma_start(wfc2_sb[:], wfc2[:])

        # Persistent state: cell state and the double-buffered hidden state.
        c_st = persist.tile([2 * H, NG], DT, tag="c")
        nc.vector.memset(c_st[:], 0.0)
        hbuf = []
        for j in range(2):
            hb = persist.tile([2 * H, NG], DT, tag=f"h{j}")
            nc.vector.memset(hb[:], 0.0)
            hbuf.append(hb)

        def xs(t):
            q = 64 * (t % XFOLD)
            return xall[q : q + KX, t // XFOLD, :]

        prev_tcs = None
        for t in range(T):
            hc = hbuf[t % 2]
            hn = hbuf[(t + 1) % 2]

            # PSUM: `start=True` clears the accumulate (has_written) bits of
            # its whole BANK, so each gate chunk gets a private 2 KB bank.
            BK = 512  # fp32 elements per PSUM bank (per partition)
            psIF = psum.tile([2 * H, 2 * BK], F32, tag="psIF")  # i @0, f @512
            psGO = psum.tile([2 * H, 2 * BK], F32, tag="psGO")  # g @0, o @512
            regions = {
                0: psIF[:, 0:NG],            # i
                1: psIF[:, BK : BK + NG],    # f
                2: psGO[:, 0:NG],            # g
                3: psGO[:, BK : BK + NG],    # o
            }

            # x/bias parts: pre-run in the PE's idle window while the
            # h-matmuls wait for h (gated by the prior step's ACT reads of
            # these tiles, which is timing-harmless).
            q = 64 * (t % XFOLD)
            for ch in CHUNKS:
                nc.tensor.matmul(
                    regions[ch],
                    wx2_sb[q : q + KX, ch, :],
                    xs(t),
                    start=True,
                    stop=False,
                )
            # Dummy matmuls keep the PE streaming from the x-parts (gated by
            # the prior step's sif read) up to the h-wave deadline, so mm_f
            # issues back-to-back (warm PE: ~107 ns instead of ~265 ns).
            if N_DUMMY and t > 0:
                scr = psum2.tile([2 * H, BK], F32, tag="scr")
                for d in range(N_DUMMY):
                    nc.tensor.matmul(
                        scr[:, 0:DUMMY_FD],
                        wx2_sb[0:KX, 0, 0:NG],
                        xall[0:KX, 0 : DUMMY_FD // NG, :],
                        start=True,
                        stop=True,
                    )
            # recurrent parts: the 4-matmul wave gating the step.
            for ch in CHUNKS:
                nc.tensor.matmul(
                    regions[ch], wh2_sb[:, ch, :], hc[:], start=False, stop=True
                )

            tg = acts.tile([2 * H, NG], DT, tag="tg")
            sif = acts.tile([2 * H, 2 * NG], DT, tag="sif")
            if pad2 and prev_tcs is not None:
                padt2 = acts.tile([2 * H, pad2], DT, tag="padt2")
                nc.scalar.activation(
                    padt2[:],
                    prev_tcs[:, 0:1].broadcast_to((2 * H, pad2)),
                    AF.Sigmoid,
                )

            nc.scalar.activation(
                sif[:],
                psIF[:].rearrange("p (b n) -> p b n", b=2)[:, :, 0:NG],
                AF.Sigmoid,
            )
            nc.scalar.activation(tg[:], regions[2], AF.Tanh)
            so = acts.tile([2 * H, NG], DT, tag="so")
            nc.scalar.activation(so[:], regions[3], AF.Sigmoid)

            si = sif[:, 0:NG]
            sf = sif[:, NG : 2 * NG]

            w = temps.tile([2 * H, NG], DT, tag="w")
            nc.vector.tensor_mul(w[:], sf, c_st[:])
            u = temps.tile([2 * H, NG], DT, tag="u")
            nc.vector.tensor_mul(u[:], si, tg[:])
            nc.vector.tensor_add(c_st[:], u[:], w[:])

            if pad1:
                padt1 = acts.tile([2 * H, pad1], DT, tag="padt1")
                nc.scalar.activation(padt1[:], so[:, 0:pad1], AF.Sigmoid)
                padt1b = acts.tile([2 * H, pad1], DT, tag="padt1b")
                nc.scalar.activation(padt1b[:], padt1[:, 0:pad1], AF.Sigmoid)
            tcs = acts.tile([2 * H, NG], DT, tag="tc")
            nc.scalar.activation(tcs[:], c_st[:], AF.Tanh)
            prev_tcs = tcs

            nc.vector.tensor_mul(hn[:], so[:], tcs[:])

        # Final FC: one matmul, both groups ([O g0 | O g1] output rows).
        hf = hbuf[T % 2]
        fc_ps = psum1.tile([2 * O, NG], F32, tag="fc")
        nc.tensor.matmul(fc_ps[:], wfc2_sb[:], hf[:], start=True, stop=True)
        fc_sb = temps.tile([2 * O, NG], F32, tag="fcsb")
        nc.vector.tensor_copy(fc_sb[:], fc_ps[:])
        nc.sync.dma_start(out[:], fc_sb[:])

    nc.compile()
    return nc


def prep_weights(W_ih, W_hh, b_ih, b_hh, W_fc, b_fc):
    bsum = (b_ih + b_hh).astype(np.float32)
    wh2 = np.zeros((2 * H, 4, 2 * H), np.float32)
    wx2 = np.zeros((KX, 4, 2 * H), np.float32)
    for ch in range(4):
        r = slice(ch * H, (ch + 1) * H)
        wh2[0:H, ch, 0:H] = W_hh[r].T
        wh2[H:, ch, H:] = W_hh[r].T
        wx2[0, ch, 0:H] = bsum[r]
        wx2[1 : 1 + I, ch, 0:H] = W_ih[r].T
        wx2[1 + I, ch, H:] = bsum[r]
        wx2[2 + I :, ch, H:] = W_ih[r].T
    wfc2 = np.zeros((2 * H, 2 * O), np.float32)
    wfc2[0:H, 0:O] = W_fc.T
    wfc2[H:, O:] = W_fc.T
    wx4 = np.zeros((128, 4, 2 * H), np.float32)
    for qq in range(XFOLD):
        wx4[64 * qq : 64 * qq + KX] = wx2
    return wh2, wx4, wfc2


def make_in_maps(x, W_ih, W_hh, b_ih, b_hh, W_fc, b_fc, T=T_FULL, use_bf16=None):
    import ml_dtypes

    if use_bf16 is None:
        use_bf16 = USE_BF16
    npdt = ml_dtypes.bfloat16 if use_bf16 else np.float32
    wh2, wx2, wfc2 = prep_weights(W_ih, W_hh, b_ih, b_hh, W_fc, b_fc)
    wh2, wx2, wfc2 = (a.astype(npdt) for a in (wh2, wx2, wfc2))
    TQ = T // XFOLD
    in_maps = []
    for core in range(NCORES):
        xc = x[core * BLOC : (core + 1) * BLOC, :T, :]  # [BLOC, T, I]
        xT = np.ascontiguousarray(xc.transpose(1, 2, 0))  # [T, I, BLOC]
        xT2 = np.empty((T, KX, NG), np.float32)
        xT2[:, 0, :] = 1.0
        xT2[:, 1 : 1 + I, :] = xT[:, :, 0:NG]
        xT2[:, 1 + I, :] = 1.0
        xT2[:, 2 + I :, :] = xT[:, :, NG : 2 * NG]
        # Fold 2 consecutive timesteps into the partition dim:
        # xq[64*q + k, u, n] = xT2[u*XFOLD + q, k, n]
        xqa = np.zeros((128, TQ, NG), np.float32)
        folded = xT2.reshape(TQ, XFOLD, KX, NG).transpose(1, 2, 0, 3)
        for qq in range(XFOLD):
            xqa[64 * qq : 64 * qq + KX] = folded[qq]
        in_maps.append(
            {"xq": xqa.astype(npdt), "wh2": wh2, "wx2": wx2, "wfc2": wfc2}
        )
    return in_maps


_CACHED_NC = None


def kernel(x, W_ih, W_hh, b_ih, b_hh, W_fc, b_fc):
    global _CACHED_NC
    x = np.asarray(x, np.float32)
    args = [np.asarray(a, np.float32) for a in (W_ih, W_hh, b_ih, b_hh, W_fc, b_fc)]
    if _CACHED_NC is None:
        _CACHED_NC = build_nc()
    nc = _CACHED_NC
    in_maps = make_in_maps(x, *args)
    res = run_bass_kernel_spmd(nc, in_maps, core_ids=list(range(NCORES)))
    b_fc = args[5]
    full = np.empty((1, B, O), np.float32)
    for core in range(NCORES):
        oc = res.results[core]["out"]  # [2*O, NG]
        for g in range(2):
            lo = core * BLOC + g * NG
            full[0, lo : lo + NG, :] = oc[g * O : (g + 1) * O].T + b_fc
    return full


# revision 24
# speedup vs baseline: 1.5196x; 1.5196x over previous
"""Dual-pipeline LSTM kernel: two 128-example half-batches (A: free cols 0:64,
B: cols 64:128 of every group) run phase-shifted so the per-engine queues stay
dense.  Engine issue order per step is hand-interleaved:
  PE:  hA-wave, hB-wave, xA(t+1), xB(t+1)
  ACT: sifA, tgA, soA, sifB, tgB, tcA, soB, tcB
  DVE: wA, uA, addA, wB, hA, uB, addB, hB
Each pipeline keeps its own state tiles and its own PSUM banks (4 tiles x 2
banks = all 8 banks; the final FC reuses pipeline A's i-bank post-loop).
"""

from contextlib import ExitStack

import numpy as np

import concourse.bass as bass
import concourse.tile as tile
from concourse import bacc, mybir
from concourse.bass_utils import run_bass_kernel_spmd

F32 = mybir.dt.float32
BF16 = mybir.dt.bfloat16
AF = mybir.ActivationFunctionType

H, I, O = 64, 4, 4
B, T_FULL = 2048, 512
NCORES = 8
BLOC = B // NCORES
NG = 128
NH = 64                     # free-dim half (per pipeline)
KX = 2 * (1 + I)
XFOLD = 2

USE_BF16 = True
CHUNKS = (1, 0, 2, 3)       # f, i, g, o


def build_nc(T=T_FULL, use_bf16=None):
    if use_bf16 is None:
        use_bf16 = USE_BF16
    DT = BF16 if use_bf16 else F32
    assert T % XFOLD == 0
    TQ = T // XFOLD
    nc = bacc.Bacc(
        "TRN2",
        target_bir_lowering=False,
        debug=False,
        enable_asserts=False,
        num_devices=NCORES,
    )

    xq = nc.dram_tensor("xq", [128, TQ, NG], DT, kind="ExternalInput")
    wh2 = nc.dram_tensor("wh2", [2 * H, 4, 2 * H], DT, kind="ExternalInput")
    wx2 = nc.dram_tensor("wx2", [128, 4, 2 * H], DT, kind="ExternalInput")
    wfc2 = nc.dram_tensor("wfc2", [2 * H, 2 * O], DT, kind="ExternalInput")
    out = nc.dram_tensor("out", [2 * O, NG], F32, kind="ExternalOutput")

    with tile.TileContext(nc) as tc, ExitStack() as ctx:
        persist = ctx.enter_context(tc.tile_pool(name="persist", bufs=1))
        acts = ctx.enter_context(tc.tile_pool(name="acts", bufs=3))
        temps = ctx.enter_context(tc.tile_pool(name="temps", bufs=3))
        psum = ctx.enter_context(tc.tile_pool(name="psum", bufs=1, space="PSUM"))

        xall = persist.tile([128, TQ, NG], DT, tag="xall")
        nc.sync.dma_start(xall[:], xq[:])
        wh2_sb = persist.tile([2 * H, 4, 2 * H], DT, tag="wh2")
        nc.sync.dma_start(wh2_sb[:], wh2[:])
        wx2_sb = persist.tile([128, 4, 2 * H], DT, tag="wx2")
        nc.scalar.dma_start(wx2_sb[:], wx2[:])
        wfc2_sb = persist.tile([2 * H, 2 * O], DT, tag="wfc2")
        nc.scalar.dma_start(wfc2_sb[:], wfc2[:])

        BK = 512
        # Per-pipeline state
        cst, hb, psIF, psGO = {}, {}, {}, {}
        for p in ("A", "B"):
            c_t = persist.tile([2 * H, NH], DT, tag=f"c{p}")
            nc.vector.memset(c_t[:], 0.0)
            cst[p] = c_t
            hb[p] = []
            for j in range(2):
                h_t = persist.tile([2 * H, NH], DT, tag=f"h{p}{j}")
                nc.vector.memset(h_t[:], 0.0)
                hb[p].append(h_t)
            ps1 = psum.tile([2 * H, 2 * BK], F32, tag=f"psIF{p}")
            psIF[p] = ps1
            ps2 = psum.tile([2 * H, 2 * BK], F32, tag=f"psGO{p}")
            psGO[p] = ps2

        def regions(p):
            return {
                0: psIF[p][:, 0:NH],
                1: psIF[p][:, BK : BK + NH],
                2: psGO[p][:, 0:NH],
                3: psGO[p][:, BK : BK + NH],
            }

        def xs(t, p):
            q = 64 * (t % XFOLD)
            lo = 0 if p == "A" else NH
            return xall[q : q + KX, t // XFOLD, lo : lo + NH]

        def x_wave(t, p):
            q = 64 * (t % XFOLD)
            reg = regions(p)
            for ch in CHUNKS:
                nc.tensor.matmul(
                    reg[ch], wx2_sb[q : q + KX, ch, :], xs(t, p),
                    start=True, stop=False,
                )

        def h_wave(t, p):
            reg = regions(p)
            for ch in CHUNKS:
                nc.tensor.matmul(
                    reg[ch], wh2_sb[:, ch, :], hb[p][t % 2][:],
                    start=False, stop=True,
                )

        # Prologue: x-parts for step 0, both pipelines.
        x_wave(0, "A")
        x_wave(0, "B")

        for t in range(T):
            # --- PE ---
            h_wave(t, "A")
            h_wave(t, "B")

            # --- ACT / DVE interleaved (program order per engine matters) ---
            sif_, tg_, so_, tcs_, w_, u_ = {}, {}, {}, {}, {}, {}
            for p in ("A", "B"):
                sif_t = acts.tile([2 * H, 2 * NH], DT, tag=f"sif{p}")
                sif_[p] = sif_t
                tg_t = acts.tile([2 * H, NH], DT, tag=f"tg{p}")
                tg_[p] = tg_t
                so_t = acts.tile([2 * H, NH], DT, tag=f"so{p}")
                so_[p] = so_t
                tcs_t = acts.tile([2 * H, NH], DT, tag=f"tc{p}")
                tcs_[p] = tcs_t
                w_t = temps.tile([2 * H, NH], DT, tag=f"w{p}")
                w_[p] = w_t
                u_t = temps.tile([2 * H, NH], DT, tag=f"u{p}")
                u_[p] = u_t

            def act_sif(p):
                nc.scalar.activation(
                    sif_[p][:],
                    psIF[p][:].rearrange("p (b n) -> p b n", b=2)[:, :, 0:NH],
                    AF.Sigmoid,
                )

            def act_tg(p):
                nc.scalar.activation(tg_[p][:], regions(p)[2], AF.Tanh)

            def act_so(p):
                nc.scalar.activation(so_[p][:], regions(p)[3], AF.Sigmoid)

            def act_tc(p):
                nc.scalar.activation(tcs_[p][:], cst[p][:], AF.Tanh)

            def dve_w(p):
                nc.vector.tensor_mul(w_[p][:], sif_[p][:, NH : 2 * NH], cst[p][:])

            def dve_u(p):
                nc.vector.tensor_mul(u_[p][:], sif_[p][:, 0:NH], tg_[p][:])

            def dve_add(p):
                nc.vector.tensor_add(cst[p][:], u_[p][:], w_[p][:])

            def dve_h(p):
                nc.vector.tensor_mul(
                    hb[p][(t + 1) % 2][:], so_[p][:], tcs_[p][:]
                )

            # ACT order: sifA tgA soA sifB tgB tcA soB tcB
            # DVE order: wA uA addA wB hA uB addB hB
            act_sif("A")
            act_tg("A")
            dve_w("A")
            dve_u("A")
            act_so("A")
            act_sif("B")
            dve_add("A")
            act_tg("B")
            dve_w("B")
            act_tc("A")
            dve_h("A")
            act_so("B")
            dve_u("B")
            dve_add("B")
            act_tc("B")
            dve_h("B")
            # x-parts for t+1 issued last (group checker wants the reads of
            # step t's banks issued before the next group opens); PE queue
            # order is unaffected.
            if t + 1 < T:
                x_wave(t + 1, "A")
                x_wave(t + 1, "B")

        # FC tail: reuse pipeline A's i-bank for the PSUM output.
        fcA = psIF["A"][0 : 2 * O, 0:NH]
        fcB = psIF["A"][0 : 2 * O, NH:NG]
        nc.tensor.matmul(fcA, wfc2_sb[:], hb["A"][T % 2][:], start=True, stop=True)
        nc.tensor.matmul(
            fcB, wfc2_sb[:], hb["B"][T % 2][:],
            start=False, stop=True, skip_group_check=True,
        )
        fc_sb = temps.tile([2 * O, NG], F32, tag="fcsb")
        nc.vector.tensor_copy(fc_sb[:], psIF["A"][0 : 2 * O, 0:NG])
        nc.sync.dma_start(out[:], fc_sb[:])

    nc.compile()
    return nc


def prep_weights(W_ih, W_hh, b_ih, b_hh, W_fc, b_fc):
    bsum = (b_ih + b_hh).astype(np.float32)
    wh2 = np.zeros((2 * H, 4, 2 * H), np.float32)
    wx2 = np.zeros((KX, 4, 2 * H), np.float32)
    for ch in range(4):
        r = slice(ch * H, (ch + 1) * H)
        wh2[0:H, ch, 0:H] = W_hh[r].T
        wh2[H:, ch, H:] = W_hh[r].T
        wx2[0, ch, 0:H] = bsum[r]
        wx2[1 : 1 + I, ch, 0:H] = W_ih[r].T
        wx2[1 + I, ch, H:] = bsum[r]
        wx2[2 + I :, ch, H:] = W_ih[r].T
    wfc2 = np.zeros((2 * H, 2 * O), np.float32)
    wfc2[0:H, 0:O] = W_fc.T
    wfc2[H:, O:] = W_fc.T
    wx4 = np.zeros((128, 4, 2 * H), np.float32)
    for qq in range(XFOLD):
        wx4[64 * qq : 64 * qq + KX] = wx2
    return wh2, wx4, wfc2


def make_in_maps(x, W_ih, W_hh, b_ih, b_hh, W_fc, b_fc, T=T_FULL, use_bf16=None):
    import ml_dtypes

    if use_bf16 is None:
        use_bf16 = USE_BF16
    npdt = ml_dtypes.bfloat16 if use_bf16 else np.float32
    wh2, wx4, wfc2 = prep_weights(W_ih, W_hh, b_ih, b_hh, W_fc, b_fc)
    wh2, wx4, wfc2 = (a.astype(npdt) for a in (wh2, wx4, wfc2))
    TQ = T // XFOLD
    in_maps = []
    for core in range(NCORES):
        xc = x[core * BLOC : (core + 1) * BLOC, :T, :]
        xT = np.ascontiguousarray(xc.transpose(1, 2, 0))  # [T, I, BLOC]
        xT2 = np.empty((T, KX, NG), np.float32)
        xT2[:, 0, :] = 1.0
        xT2[:, 1 : 1 + I, :] = xT[:, :, 0:NG]
        xT2[:, 1 + I, :] = 1.0
        xT2[:, 2 + I :, :] = xT[:, :, NG : 2 * NG]
        xqa = np.zeros((128, TQ, NG), np.float32)
        folded = xT2.reshape(TQ, XFOLD, KX, NG).transpose(1, 2, 0, 3)
        for qq in range(XFOLD):
            xqa[64 * qq : 64 * qq + KX] = folded[qq]
        in_maps.append(
            {"xq": xqa.astype(npdt), "wh2": wh2, "wx2": wx4, "wfc2": wfc2}
        )
    return in_maps


_CACHED_NC = None


def kernel(x, W_ih, W_hh, b_ih, b_hh, W_fc, b_fc):
    global _CACHED_NC
    x = np.asarray(x, np.float32)
    args = [np.asarray(a, np.float32) for a in (W_ih, W_hh, b_ih, b_hh, W_fc, b_fc)]
    if _CACHED_NC is None:
        _CACHED_NC = build_nc()
    nc = _CACHED_NC
    in_maps = make_in_maps(x, *args)
    res = run_bass_kernel_spmd(nc, in_maps, core_ids=list(range(NCORES)))
    b_fc = args[5]
    full = np.empty((1, B, O), np.float32)
    for core in range(NCORES):
        oc = res.results[core]["out"]  # [2*O, NG]
        for g in range(2):
            lo = core * BLOC + g * NG
            full[0, lo : lo + NG, :] = oc[g * O : (g + 1) * O].T + b_fc
    return full
